# Initial kernel scaffold
#
"""Trainium2 Bass kernel for nn_BaseDecoder (6-layer transformer decoder).

Sharding: data-parallel over batch, 8 NeuronCores x 4 batch elements.
Per-core layout: activations feature-major ("xT": [E partitions, tokens free]).
All matmuls fp16 w/ fp32 PSUM; layer-1 self-attn q/k/scores emulate fp32 via
hi/lo fp16 splits (scores there are ~N(0,590) and argmax-sensitive).
Attention scores are computed transposed ([k, q]) so the gathered relative
bias + causal mask (fp16, pre-scaled by 8, -480 masked fill) streams in
matching layout; softmax normalization: row-sum via ones-matmul -> reciprocal
on the [1, q] row -> PE broadcast -> multiplied into P before attn@V.
LayerNorm: partition sums via ones-matmuls, row math, PE broadcast, in-place.
"""
import sys
sys.path.insert(0, '/opt/trn_rl_repo')

import numpy as np
import concourse.bass as bass
import concourse.bacc as bacc
import concourse.mybir as mybir
import concourse.tile as tile
from concourse.bass_utils import run_bass_kernel_spmd
from contextlib import ExitStack

F32 = mybir.dt.float32
F16 = mybir.dt.float16
I16 = mybir.dt.int16
AF = mybir.ActivationFunctionType
ALU = mybir.AluOpType

B, S, M, E, H, F, L, V = 32, 256, 128, 1024, 16, 4096, 6, 200
DH = E // H
NCORES = 8
BL = B // NCORES
TOK = BL * S          # 1024
EC = E // 128         # 8
FC = F // 128         # 32
LN_EPS = 1e-5
MASK8 = -30000.0      # masked-entry fill (x8 units); kills exp even vs L1 max gap
VP = 256

_built = {}
_last_res = {}


def build_nc():
    nc = bacc.Bacc("TRN2", target_bir_lowering=False, debug=False)
    din = {}

    def inp(name, shape, dtype):
        din[name] = nc.dram_tensor(name, list(shape), dtype, kind="ExternalInput")

    inp("tokwT", (E, V), F32)
    inp("posencT", (E, S), F32)
    inp("seq_idx", (128, TOK // 16), I16)
    inp("bias_tab8", (128, 400), F32)
    inp("bias_idx", (BL, 128, 8192 // 16), I16)
    inp("bias_mask8", (128, 8192), F32)
    inp("mask_qk", (2, 128, S), F32)
    inp("identity", (128, 128), F32)
    inp("WqkvT", (L, 3 * EC, EC, 128, 128), F16)
    inp("Wqk_lo", (2 * EC, EC, 128, 128), F16)
    inp("WoT", (L, EC, EC, 128, 128), F16)
    inp("cWqkvT", (L, 3 * EC, EC, 128, 128), F16)
    inp("cWoT", (L, EC, EC, 128, 128), F16)
    inp("W1T", (L, FC, EC, 128, 128), F16)
    inp("W2T", (L, EC, FC, 128, 128), F16)
    inp("genT_hi", (EC, 128, VP), F16)
    inp("genT_lo", (EC, 128, VP), F16)
    inp("memT", (E, BL * M), F16)
    inp("WvT_mov", (L, 2, 128, EC * 512), F16)
    inp("cWvT_mov", (L, 2, 128, EC * 512), F16)
    out_t = nc.dram_tensor("out", [BL, S, V], F32, kind="ExternalOutput")
    bias_scr = nc.dram_tensor("bias_scr", [BL, 128, 8192], F16)

    with tile.TileContext(nc) as tc, ExitStack() as ctx:
        big = ctx.enter_context(tc.tile_pool(name="big", bufs=1))
        wpool = ctx.enter_context(tc.tile_pool(name="wp", bufs=2))
        sm = ctx.enter_context(tc.tile_pool(name="sm", bufs=1))
        ph = ctx.enter_context(tc.tile_pool(name="ph", bufs=2))   # per-head small tiles
        bias_p = ctx.enter_context(tc.tile_pool(name="biasp", bufs=2))
        wp2 = ctx.enter_context(tc.tile_pool(name="wp2", bufs=1))
        pgemm = ctx.enter_context(tc.tile_pool(name="pg", bufs=3, space="PSUM"))
        psT = ctx.enter_context(tc.tile_pool(name="psT", bufs=2, space="PSUM"))
        prow = ctx.enter_context(tc.tile_pool(name="prow", bufs=1, space="PSUM"))
        pbz = ctx.enter_context(tc.tile_pool(name="pbz", bufs=1, space="PSUM"))
        pout = ctx.enter_context(tc.tile_pool(name="pout", bufs=1, space="PSUM"))

        # ---------------- constants ----------------
        ident = big.tile([128, 128], F32, tag="ident")
        nc.sync.dma_start(ident[:], din["identity"][:])
        ones_col = big.tile([128, 1], F16, tag="ones_col")
        nc.vector.memset(ones_col[:], 1.0)
        ones_row = big.tile([1, 128], F16, tag="ones_row")
        nc.vector.memset(ones_row[:], 1.0)
        epsc = big.tile([128, 1], F32, tag="epsc")
        nc.vector.memset(epsc[:], LN_EPS)
        maskqk = big.tile([128, 2 * S], F32, tag="maskqk")
        nc.sync.dma_start(maskqk[:, 0:S], din["mask_qk"][0])
        nc.sync.dma_start(maskqk[:, S:2 * S], din["mask_qk"][1])
        memsb = big.tile([128, EC * 512], F16, tag="memsb")
        nc.sync.dma_start(memsb[:], din["memT"][:].rearrange("(ec p) t -> p ec t", p=128))

        # ---------------- embeddings ----------------
        A = big.tile([128, EC * TOK], F32, tag="A")
        tokw = big.tile([128, EC * V], F32, tag="qkA", name="tokw")
        nc.sync.dma_start(tokw[:], din["tokwT"][:].rearrange("(ec p) v -> p ec v", p=128))
        sidx = big.tile([128, TOK // 16], I16, tag="sidx")
        nc.sync.dma_start(sidx[:], din["seq_idx"][:])
        posenc = big.tile([128, EC * S], F32, tag="qkB", name="posenc")
        nc.sync.dma_start(posenc[:], din["posencT"][:].rearrange("(ec p) s -> p ec s", p=128))
        for ec in range(EC):
            nc.gpsimd.ap_gather(A[:, ec * TOK:(ec + 1) * TOK], tokw[:, ec * V:(ec + 1) * V],
                                sidx[:], channels=128, num_elems=V, d=1, num_idxs=TOK)
        for ec in range(EC):
            for b in range(BL):
                sl = A[:, ec * TOK + b * S: ec * TOK + (b + 1) * S]
                nc.vector.tensor_tensor(sl, sl, posenc[:, ec * S:(ec + 1) * S], op=ALU.add)

        # ---------------- bias build ----------------
        btab = big.tile([128, 400], F32, tag="btab")
        nc.sync.dma_start(btab[:], din["bias_tab8"][:])
        bmask = big.tile([128, 8192], F32, tag="qkB", name="bmask")
        nc.sync.dma_start(bmask[:], din["bias_mask8"][:])
        for b in range(BL):
            bidx = sm.tile([128, 512], I16, tag="bidx")
            nc.sync.dma_start(bidx[:], din["bias_idx"][b])
            graw = big.tile([128, 8192], F32, tag="qkA", name=f"graw{b}")
            nc.gpsimd.ap_gather(graw[:], btab[:], bidx[:], channels=128,
                                num_elems=400, d=1, num_idxs=8192)
            g16 = big.tile([128, 8192], F16, tag="vtok", name=f"g16_{b}")
            nc.vector.tensor_tensor(g16[:], graw[:], bmask[:], op=ALU.add)
            nc.sync.dma_start(bias_scr[b], g16[:])

        # -------------- persistent buffers --------------
        B16 = big.tile([128, EC * TOK], F16, tag="B16")

        _nn = [0]

        def _named(tag, shape, dtype):
            _nn[0] += 1
            return big.tile(shape, dtype, tag=tag, name=f"{tag}_{_nn[0]}")

        def new_qkA(dtype, n):
            return _named("qkA", [128, n], dtype)

        def new_qkB(dtype, n):
            return _named("qkB", [128, n], dtype)

        def new_alo():
            return _named("vtok", [128, EC * TOK], F16)

        def new_qcT():
            return _named("qkA", [128, EC * TOK], F16)

        def new_vtok():
            return _named("vtok", [128, EC * TOK], F16)

        # -------------- helpers --------------
        def hilo_row(dh_, dl_, src, n):
            nc.vector.tensor_copy(dh_[:, 0:n], src[:, 0:n])
            nc.vector.tensor_tensor(dl_[:, 0:n], src[:, 0:n], dh_[:, 0:n], op=ALU.subtract)

        def bcast_hilo(ps, rh, rl, n):
            nc.tensor.matmul(ps[:, 0:n], ones_row[:], rh[:, 0:n], start=True, stop=False)
            nc.tensor.matmul(ps[:, 0:n], ones_row[:], rl[:, 0:n], start=False, stop=True)

        def layernorm():
            """in-place LN of A; refresh B16."""
            a16 = _named("qkA", [128, EC * TOK], F16)
            sq = _named("vtok", [128, EC * TOK], F16)
            nc.vector.tensor_copy(a16[:], A[:])
            nc.scalar.activation(sq[:], A[:], AF.Square)
            negm = sm.tile([1, TOK], F32, tag="ln_negm")
            rr = sm.tile([1, TOK], F32, tag="ln_rr")
            for tkc in range(2):
                o = tkc * 512
                s1 = prow.tile([1, 512], F32, tag="row")
                for ec in range(EC):
                    nc.tensor.matmul(s1[:], ones_col[:], a16[:, ec * TOK + o: ec * TOK + o + 512],
                                     start=(ec == 0), stop=(ec == EC - 1))
                nc.scalar.activation(negm[:, o:o + 512], s1[:], AF.Copy, scale=-1.0 / E)
                s2 = prow.tile([1, 512], F32, tag="row")
                for ec in range(EC):
                    nc.tensor.matmul(s2[:], ones_col[:], sq[:, ec * TOK + o: ec * TOK + o + 512],
                                     start=(ec == 0), stop=(ec == EC - 1))
                v1 = sm.tile([1, 512], F32, tag="ln_v1")
                nc.scalar.activation(v1[:], s2[:], AF.Copy, scale=1.0 / E)
                m2 = sm.tile([1, 512], F32, tag="ln_m2")
                nc.vector.tensor_tensor(m2[:], negm[:, o:o + 512], negm[:, o:o + 512], op=ALU.mult)
                nc.vector.tensor_tensor(v1[:], v1[:], m2[:], op=ALU.subtract)
                sd = sm.tile([1, 512], F32, tag="ln_sd")
                nc.scalar.activation(sd[:], v1[:], AF.Sqrt, bias=epsc[0:1, :])
                nc.vector.reciprocal(rr[:, o:o + 512], sd[:])
            nmh = sm.tile([1, TOK], F16, tag="ln_nmh")
            rrh = sm.tile([1, TOK], F16, tag="ln_rrh")
            nc.vector.tensor_copy(nmh[:], negm[:])
            nc.vector.tensor_copy(rrh[:], rr[:])
            for tkc in range(2):
                o = tkc * 512
                mb = pgemm.tile([128, 512], F32, tag="g")
                rb = pgemm.tile([128, 512], F32, tag="g")
                nc.tensor.matmul(mb[:], ones_row[:], nmh[:, o:o + 512])
                nc.tensor.matmul(rb[:], ones_row[:], rrh[:, o:o + 512])
                for ec in range(EC):
                    sl = A[:, ec * TOK + o: ec * TOK + o + 512]
                    nc.vector.tensor_tensor(sl, sl, mb[:], op=ALU.add)
                    nc.vector.tensor_tensor(sl, sl, rb[:], op=ALU.mult)
                    nc.vector.tensor_copy(B16[:, ec * TOK + o: ec * TOK + o + 512], sl)

        def gemm_oc_tok(dst, wdram, l_idx, octile0, n_octiles, mov, mov_lo=None,
                        w_lo=None, wlo_octile0=0, dst_hilo=False, dst_off=0):
            """dst[oc_tile*TOK + tok] = W.x ; stat = weight tiles, mov feature-major."""
            for mt in range(n_octiles):
                wt = wpool.tile([128, EC * 128], F16, tag="wload")
                src = wdram[l_idx, octile0 + mt] if l_idx is not None else wdram[octile0 + mt]
                nc.sync.dma_start(wt[:], src.rearrange("kc a b -> a kc b"))
                wlt = None
                if w_lo is not None:
                    wlt = wp2.tile([128, EC * 128], F16, tag="w2load")
                    nc.sync.dma_start(wlt[:], w_lo[wlo_octile0 + mt].rearrange("kc a b -> a kc b"))
                for tkc in range(2):
                    o = tkc * 512
                    ps = pgemm.tile([128, 512], F32, tag="g")
                    nmm = EC * (3 if w_lo is not None else 1)
                    i = 0
                    for kc in range(EC):
                        mv = mov[:, kc * TOK + o: kc * TOK + o + 512]
                        nc.tensor.matmul(ps[:], wt[:, kc * 128:(kc + 1) * 128], mv,
                                         start=(i == 0), stop=(i == nmm - 1)); i += 1
                        if w_lo is not None:
                            mvl = mov_lo[:, kc * TOK + o: kc * TOK + o + 512]
                            nc.tensor.matmul(ps[:], wt[:, kc * 128:(kc + 1) * 128], mvl,
                                             start=False, stop=(i == nmm - 1)); i += 1
                            nc.tensor.matmul(ps[:], wlt[:, kc * 128:(kc + 1) * 128], mv,
                                             start=False, stop=(i == nmm - 1)); i += 1
                    if dst_hilo:
                        hi_sl = dst[:, mt * TOK + o: mt * TOK + o + 512]
                        lo_sl = dst[:, 8192 + mt * TOK + o: 8192 + mt * TOK + o + 512]
                        nc.vector.tensor_copy(hi_sl, ps[:])
                        nc.vector.tensor_tensor(lo_sl, ps[:], hi_sl, op=ALU.subtract)
                    else:
                        nc.vector.tensor_copy(dst[:, dst_off + mt * TOK + o: dst_off + mt * TOK + o + 512], ps[:])

        def residual_gemm(wdram, l_idx, mov):
            """A += W.mov  (Wo / cWo / ffn2-style: E out-tiles)"""
            for mt in range(EC):
                wt = wpool.tile([128, EC * 128], F16, tag="wload")
                nc.sync.dma_start(wt[:], wdram[l_idx, mt].rearrange("kc a b -> a kc b"))
                for tkc in range(2):
                    o = tkc * 512
                    ps = pgemm.tile([128, 512], F32, tag="g")
                    for kc in range(EC):
                        nc.tensor.matmul(ps[:], wt[:, kc * 128:(kc + 1) * 128],
                                         mov[:, kc * TOK + o: kc * TOK + o + 512],
                                         start=(kc == 0), stop=(kc == EC - 1))
                    sl = A[:, mt * TOK + o: mt * TOK + o + 512]
                    nc.vector.tensor_tensor(sl, sl, ps[:], op=ALU.add)

        # ================== layers ==================
        for l in range(L):
            first = (l == 0)
            # ---------- self-attention: q/k/v projections ----------
            if first:
                XHI = B16
                XLO = new_alo()
                nc.vector.tensor_copy(XHI[:], A[:])
                nc.vector.tensor_tensor(XLO[:], A[:], XHI[:], op=ALU.subtract)
                qT = new_qkA(F16, 2 * EC * TOK)
                kT = new_qkB(F16, 2 * EC * TOK)
                gemm_oc_tok(qT, din["WqkvT"], 0, 0, EC, XHI, mov_lo=XLO,
                            w_lo=din["Wqk_lo"], wlo_octile0=0, dst_hilo=True)
                gemm_oc_tok(kT, din["WqkvT"], 0, EC, EC, XHI, mov_lo=XLO,
                            w_lo=din["Wqk_lo"], wlo_octile0=EC, dst_hilo=True)
            else:
                qT = new_qkA(F16, EC * TOK)
                kT = new_qkB(F16, EC * TOK)
                gemm_oc_tok(qT, din["WqkvT"], l, 0, EC, B16)
                gemm_oc_tok(kT, din["WqkvT"], l, EC, EC, B16)
            # v gemm: out [tok, oc]; stat = B16 token tiles, mov = WvT columns
            VT = new_vtok()
            for occ in range(2):
                wv = wpool.tile([128, EC * 512], F16, tag="wvload")
                nc.sync.dma_start(wv[:], din["WvT_mov"][l, occ])
                for tt in range(EC):
                    ps = pgemm.tile([128, 512], F32, tag="g")
                    for kc in range(EC):
                        nc.tensor.matmul(ps[:], B16[:, kc * TOK + tt * 128: kc * TOK + tt * 128 + 128],
                                         wv[:, kc * 512:(kc + 1) * 512],
                                         start=(kc == 0), stop=(kc == EC - 1))
                    nc.vector.tensor_copy(VT[:, tt * E + occ * 512: tt * E + occ * 512 + 512], ps[:])

            # ---------- L1: per-(bh,qc) masked max ----------
            if first:
                negMb0 = sm.tile([128, 64], F32, tag="negMb0")
                negMb1 = sm.tile([128, 64], F32, tag="negMb1")
                negMb = [negMb0, negMb1]
                for b in range(BL):
                    for h in range(H):
                        bh = b * H + h
                        e2, off = h // 2, (h % 2) * 64
                        qh = qT[off:off + 64, e2 * TOK + b * S: e2 * TOK + (b + 1) * S]
                        ql = qT[off:off + 64, 8192 + e2 * TOK + b * S: 8192 + e2 * TOK + (b + 1) * S]
                        kh = kT[off:off + 64, e2 * TOK + b * S: e2 * TOK + (b + 1) * S]
                        kl = kT[off:off + 64, 8192 + e2 * TOK + b * S: 8192 + e2 * TOK + (b + 1) * S]
                        for qc in range(2):
                            ps = psT.tile([128, S], F32, tag="sT")
                            nc.tensor.matmul(ps[:], qh[:, qc * 128:(qc + 1) * 128], kh[:],
                                             start=True, stop=False)
                            nc.tensor.matmul(ps[:], qh[:, qc * 128:(qc + 1) * 128], kl[:],
                                             start=False, stop=False)
                            nc.tensor.matmul(ps[:], ql[:, qc * 128:(qc + 1) * 128], kh[:],
                                             start=False, stop=True)
                            scr = ph.tile([128, S], F32, tag="ttr_scr")
                            nc.vector.tensor_tensor(scr[:], ps[:],
                                                    maskqk[:, qc * S:(qc + 1) * S],
                                                    op=ALU.add)
                            nc.vector.tensor_reduce(negMb[qc][:, bh:bh + 1], scr[:],
                                                    axis=mybir.AxisListType.X,
                                                    op=ALU.max)
                negMT = sm.tile([64, S], F32, tag="negMT")
                for qc in range(2):
                    pt = pout.tile([64, 256], F32, tag="aout")
                    nc.tensor.transpose(pt[0:64, 0:128], negMb[qc][:], ident[:])
                    nc.vector.tensor_copy(negMT[:, qc * 128:(qc + 1) * 128], pt[0:64, 0:128])
                negMTh2 = sm.tile([64, 256], F16, tag="negMTh2")
                negMTl2 = sm.tile([64, 256], F16, tag="negMTl2")
                hilo_row(negMTh2, negMTl2, negMT, 256)

            # ---------- self-attention core ----------
            AO = B16   # attn output overwrites B16 (last gemm consumer done)
            for b in range(BL):
                for h in range(H):
                    bh = b * H + h
                    e2, off = h // 2, (h % 2) * 64
                    qsl = qT[off:off + 64, e2 * TOK + b * S: e2 * TOK + (b + 1) * S]
                    ksl = kT[off:off + 64, e2 * TOK + b * S: e2 * TOK + (b + 1) * S]
                    btile = bias_p.tile([128, 512], F16, tag="bias")
                    for kc in range(2):
                        src = bias_scr[b, 64 * kc + h: 64 * kc + h + 49: 16, :]
                        nc.sync.dma_start(
                            btile[:, kc * S:(kc + 1) * S],
                            src.rearrange("g (k q) -> g k q", q=S))
                    if first:
                        nmrh = ph.tile([1, S], F16, tag="nmrh")
                        nmrl = ph.tile([1, S], F16, tag="nmrl")
                        nc.sync.dma_start(nmrh[:], negMTh2[bh:bh + 1, :])
                        nc.sync.dma_start(nmrl[:], negMTl2[bh:bh + 1, :])
                        qh = qT[off:off + 64, e2 * TOK + b * S: e2 * TOK + (b + 1) * S]
                        ql = qT[off:off + 64, 8192 + e2 * TOK + b * S: 8192 + e2 * TOK + (b + 1) * S]
                        kh = kT[off:off + 64, e2 * TOK + b * S: e2 * TOK + (b + 1) * S]
                        kl = kT[off:off + 64, 8192 + e2 * TOK + b * S: 8192 + e2 * TOK + (b + 1) * S]
                        bz = pbz.tile([128, S], F32, tag="bz")
                        bcast_hilo(bz, nmrh[:], nmrl[:], S)
                    PT = ph.tile([128, 2 * S], F16, tag="PT")
                    for kc in range(2):
                        ps = psT.tile([128, S], F32, tag="sT")
                        if first:
                            nc.tensor.matmul(ps[:], kh[:, kc * 128:(kc + 1) * 128], qh[:],
                                             start=True, stop=False)
                            nc.tensor.matmul(ps[:], kh[:, kc * 128:(kc + 1) * 128], ql[:],
                                             start=False, stop=False)
                            nc.tensor.matmul(ps[:], kl[:, kc * 128:(kc + 1) * 128], qh[:],
                                             start=False, stop=True)
                        else:
                            nc.tensor.matmul(ps[:], ksl[:, kc * 128:(kc + 1) * 128], qsl)
                        t1 = ph.tile([128, S], F32 if first else F16, tag="t1")
                        nc.vector.tensor_tensor(t1[:], ps[:], btile[:, kc * S:(kc + 1) * S],
                                                op=ALU.add)
                        if first:
                            nc.vector.tensor_tensor(t1[:], t1[:], bz[:], op=ALU.subtract)
                        nc.scalar.activation(PT[:, kc * S:(kc + 1) * S], t1[:], AF.Exp,
                                             scale=0.125)
                    zr = prow.tile([1, S], F32, tag="row")
                    for kc in range(2):
                        nc.tensor.matmul(zr[:], ones_col[:], PT[:, kc * S:(kc + 1) * S],
                                         start=(kc == 0), stop=(kc == 1))
                    rz = ph.tile([1, S], F32, tag="rz")
                    nc.vector.reciprocal(rz[:], zr[:])
                    rzh = ph.tile([1, S], F16, tag="rzh")
                    rzl = ph.tile([1, S], F16, tag="rzl")
                    hilo_row(rzh, rzl, rz, S)
                    zb = pbz.tile([128, S], F32, tag="bz")
                    bcast_hilo(zb, rzh, rzl, S)
                    po = pout.tile([64, S], F32, tag="aout")
                    for kc in range(2):
                        pn = ph.tile([128, S], F16, tag="pn")
                        nc.vector.tensor_tensor(pn[:], PT[:, kc * S:(kc + 1) * S], zb[:],
                                                op=ALU.mult)
                        nc.tensor.matmul(po[:], VT[:, (2 * b + kc) * E + h * 64: (2 * b + kc) * E + h * 64 + 64],
                                         pn[:], start=(kc == 0), stop=(kc == 1))
                    nc.vector.tensor_copy(
                        AO[(h % 2) * 64:(h % 2) * 64 + 64, (h // 2) * TOK + b * S:(h // 2) * TOK + (b + 1) * S],
                        po[:])
            residual_gemm(din["WoT"], l, AO)
            layernorm()

            # ---------- cross-attention ----------
            qcT = new_qcT()
            gemm_oc_tok(qcT, din["cWqkvT"], l, 0, EC, B16)
            KV = new_vtok()     # [:, :4096] = kcT (oc x bm), [:, 4096:] = vc (bm x oc)
            for mt in range(EC):
                wt = wpool.tile([128, EC * 128], F16, tag="wload")
                nc.sync.dma_start(wt[:], din["cWqkvT"][l, EC + mt].rearrange("kc a b -> a kc b"))
                ps = pgemm.tile([128, 512], F32, tag="g")
                for kc in range(EC):
                    nc.tensor.matmul(ps[:], wt[:, kc * 128:(kc + 1) * 128],
                                     memsb[:, kc * 512:(kc + 1) * 512],
                                     start=(kc == 0), stop=(kc == EC - 1))
                nc.vector.tensor_copy(KV[:, mt * 512:(mt + 1) * 512], ps[:])
            for occ in range(2):
                wv = wpool.tile([128, EC * 512], F16, tag="wvload", name=f"cwv_{l}_{occ}")
                nc.sync.dma_start(wv[:], din["cWvT_mov"][l, occ])
                for bt in range(BL):
                    ps = pgemm.tile([128, 512], F32, tag="g")
                    for kc in range(EC):
                        nc.tensor.matmul(ps[:], memsb[:, kc * 512 + bt * 128: kc * 512 + bt * 128 + 128],
                                         wv[:, kc * 512:(kc + 1) * 512],
                                         start=(kc == 0), stop=(kc == EC - 1))
                    nc.vector.tensor_copy(KV[:, 4096 + bt * 1024 + occ * 512: 4096 + bt * 1024 + occ * 512 + 512],
                                          ps[:])
            AO = B16
            for b in range(BL):
                for h in range(H):
                    e2, off = h // 2, (h % 2) * 64
                    ps = psT.tile([128, S], F32, tag="sT")
                    nc.tensor.matmul(ps[:], KV[off:off + 64, e2 * 512 + b * 128: e2 * 512 + (b + 1) * 128],
                                     qcT[off:off + 64, e2 * TOK + b * S: e2 * TOK + (b + 1) * S])
                    Ec = ph.tile([128, S], F16, tag="Ec")
                    nc.scalar.activation(Ec[:], ps[:], AF.Exp, scale=0.125)
                    zr = prow.tile([1, S], F32, tag="row")
                    nc.tensor.matmul(zr[:], ones_col[:], Ec[:])
                    rz = ph.tile([1, S], F32, tag="rz")
                    nc.vector.reciprocal(rz[:], zr[:])
                    rzh = ph.tile([1, S], F16, tag="rzh")
                    rzl = ph.tile([1, S], F16, tag="rzl")
                    hilo_row(rzh, rzl, rz, S)
                    zb = pbz.tile([128, S], F32, tag="bz")
                    bcast_hilo(zb, rzh, rzl, S)
                    pn = ph.tile([128, S], F16, tag="pn")
                    nc.vector.tensor_tensor(pn[:], Ec[:], zb[:], op=ALU.mult)
                    po = pout.tile([64, S], F32, tag="aout")
                    nc.tensor.matmul(po[:], KV[:, 4096 + b * 1024 + h * 64: 4096 + b * 1024 + h * 64 + 64],
                                     pn[:])
                    nc.vector.tensor_copy(
                        AO[off:off + 64, e2 * TOK + b * S: e2 * TOK + (b + 1) * S], po[:])
            residual_gemm(din["cWoT"], l, AO)
            layernorm()

            # ---------- FFN ----------
            h1a = new_qkA(F16, 16 * TOK)
            h1b = new_qkB(F16, 16 * TOK)

            def h1sl(fc, o):
                t = h1a if fc < 16 else h1b
                return t[:, (fc % 16) * TOK + o: (fc % 16) * TOK + o + 512]

            for fc in range(FC):
                wt = wpool.tile([128, EC * 128], F16, tag="wload")
                nc.sync.dma_start(wt[:], din["W1T"][l, fc].rearrange("kc a b -> a kc b"))
                for tkc in range(2):
                    o = tkc * 512
                    ps = pgemm.tile([128, 512], F32, tag="g")
                    for kc in range(EC):
                        nc.tensor.matmul(ps[:], wt[:, kc * 128:(kc + 1) * 128],
                                         B16[:, kc * TOK + o: kc * TOK + o + 512],
                                         start=(kc == 0), stop=(kc == EC - 1))
                    nc.scalar.activation(h1sl(fc, o), ps[:], AF.Gelu)
            for mt in range(EC):
                w2a = wp2.tile([128, 16 * 128], F16, tag="w2load", name=f"w2a_{l}_{mt}")
                nc.sync.dma_start(w2a[:], din["W2T"][l, mt, 0:16].rearrange("kc a b -> a kc b"))
                w2b = wp2.tile([128, 16 * 128], F16, tag="w2loadb", name=f"w2b_{l}_{mt}")
                nc.sync.dma_start(w2b[:], din["W2T"][l, mt, 16:32].rearrange("kc a b -> a kc b"))
                for tkc in range(2):
                    o = tkc * 512
                    ps = pgemm.tile([128, 512], F32, tag="g")
                    for fc in range(FC):
                        w2t = w2a if fc < 16 else w2b
                        nc.tensor.matmul(ps[:], w2t[:, (fc % 16) * 128:((fc % 16) + 1) * 128],
                                         h1sl(fc, o),
                                         start=(fc == 0), stop=(fc == FC - 1))
                    sl = A[:, mt * TOK + o: mt * TOK + o + 512]
                    nc.vector.tensor_tensor(sl, sl, ps[:], op=ALU.add)
            layernorm()

        # ---------------- final LN + generator ----------------
        layernorm()
        XLO = new_alo()
        nc.vector.tensor_tensor(XLO[:], A[:], B16[:], op=ALU.subtract)
        genh = _named("qkA", [128, EC * VP], F16)
        genl = _named("qkB", [128, EC * VP], F16)
        nc.sync.dma_start(genh[:], din["genT_hi"][:].rearrange("ec a b -> a ec b"))
        nc.sync.dma_start(genl[:], din["genT_lo"][:].rearrange("ec a b -> a ec b"))
        for tt in range(EC):
            ps = pgemm.tile([128, 512], F32, tag="g")
            n3 = 3 * EC
            i = 0
            for kc in range(EC):
                sth = B16[:, kc * TOK + tt * 128: kc * TOK + tt * 128 + 128]
                stl = XLO[:, kc * TOK + tt * 128: kc * TOK + tt * 128 + 128]
                mvh = genh[:, kc * VP:(kc + 1) * VP]
                mvl = genl[:, kc * VP:(kc + 1) * VP]
                nc.tensor.matmul(ps[:, 0:VP], sth, mvh, start=(i == 0), stop=(i == n3 - 1)); i += 1
                nc.tensor.matmul(ps[:, 0:VP], sth, mvl, start=False, stop=(i == n3 - 1)); i += 1
                nc.tensor.matmul(ps[:, 0:VP], stl, mvh, start=False, stop=(i == n3 - 1)); i += 1
            osb = bias_p.tile([128, VP], F32, tag="bias")
            nc.vector.tensor_copy(osb[:], ps[:, 0:VP])
            b0, s0 = (tt * 128) // S, (tt * 128) % S
            nc.sync.dma_start(out_t[b0, s0:s0 + 128, 0:V], osb[:, 0:V])

    nc.compile()
    return nc


# ================= host side =================

def _posenc_np():
    den = np.exp(-np.arange(0, E, 2, dtype=np.float32) *
                 np.float32(np.log(10000.0)) / np.float32(E)).astype(np.float32)
    pos = np.arange(S, dtype=np.float32)[:, None]
    pe = np.zeros((S, E), np.float32)
    pe[:, 0::2] = np.sin(pos * den)
    pe[:, 1::2] = np.cos(pos * den)
    return pe


def _tile_w(wT, dtype=np.float16):
    """[K, Mo] -> [Mo/128, K/128, 128, 128]"""
    K, Mo = wT.shape
    return np.ascontiguousarray(
        wT.reshape(K // 128, 128, Mo // 128, 128).transpose(2, 0, 1, 3)).astype(dtype)


def _wrap16(flat):
    return np.ascontiguousarray(flat.reshape(-1, 16).T)


def kernel(**inputs):
    inputs = {k: np.asarray(v) for k, v in inputs.items()}
    seqs = inputs['sequences'].astype(np.int64)
    dist = inputs['distance_squares'].astype(np.int64)
    iso = inputs['isopen_squares'].astype(np.int64)
    memory = inputs['memory'].astype(np.float32)
    tok_w = inputs['tok_emb_w'].astype(np.float32)
    dist_w = inputs['dist_emb_w'].astype(np.float32)
    iso_w = inputs['iso_emb_w'].astype(np.float32)

    if 'nc' not in _built:
        _built['nc'] = build_nc()
    nc = _built['nc']

    # ---- shared (replicated) host tensors ----
    shared = {}
    shared['tokwT'] = np.ascontiguousarray((tok_w * np.float32(np.sqrt(E))).T)
    shared['posencT'] = np.ascontiguousarray(_posenc_np().T)
    tab = np.concatenate([dist_w + iso_w[0], dist_w + iso_w[1]], axis=0)  # [400, 16]
    shared['bias_tab8'] = np.tile(np.ascontiguousarray((8.0 * tab).T), (8, 1)).astype(np.float32)
    # bias mask in gather layout: row 16g+h covers j = g*8192 + i, j = k*256+q
    jj = (np.arange(8)[:, None] * 8192 + np.arange(8192)[None, :])  # [8, 8192]
    kk, qq = jj // S, jj % S
    mrow = np.where(kk > qq, np.float32(MASK8), np.float32(0.0))    # [8, 8192]
    shared['bias_mask8'] = np.repeat(mrow, 16, axis=0).astype(np.float32)
    mq = np.zeros((2, 128, S), np.float32)
    for qc in range(2):
        qv = qc * 128 + np.arange(128)[:, None]
        mq[qc] = np.where(np.arange(S)[None, :] > qv, np.float32(-1e30), np.float32(0.0))
    shared['mask_qk'] = mq
    shared['identity'] = np.eye(128, dtype=np.float32)

    Wqkv_s = inputs['Wqkv_s'].astype(np.float32)
    shared['WqkvT'] = np.stack([_tile_w(Wqkv_s[l].T) for l in range(L)])
    qkT0 = Wqkv_s[0, :2 * E].T  # [E, 2E] f32
    hi = qkT0.astype(np.float16)
    shared['Wqk_lo'] = _tile_w((qkT0 - hi.astype(np.float32)))
    shared['WoT'] = np.stack([_tile_w(inputs['Wo_s'][l].T) for l in range(L)])
    Wqkv_c = inputs['Wqkv_c'].astype(np.float32)
    shared['cWqkvT'] = np.stack([_tile_w(Wqkv_c[l].T) for l in range(L)])
    shared['cWoT'] = np.stack([_tile_w(inputs['Wo_c'][l].T) for l in range(L)])
    def _vmov(Wqkv_f32):
        out = np.zeros((L, 2, 128, EC * 512), np.float16)
        for l in range(L):
            WvT = Wqkv_f32[l, 2 * E:3 * E].T.astype(np.float16)
            for occ in range(2):
                out[l, occ] = WvT.reshape(EC, 128, E)[:, :, occ * 512:(occ + 1) * 512]\
                    .transpose(1, 0, 2).reshape(128, EC * 512)
        return out
    shared['WvT_mov'] = _vmov(Wqkv_s)
    shared['cWvT_mov'] = _vmov(Wqkv_c)
    shared['W1T'] = np.stack([_tile_w(inputs['W1'][l].T) for l in range(L)])
    shared['W2T'] = np.stack([_tile_w(inputs['W2'][l].T) for l in range(L)])
    gpad = np.zeros((E, VP), np.float32)
    gpad[:, :V] = inputs['gen_w'].astype(np.float32).T
    gh = gpad.astype(np.float16)
    shared['genT_hi'] = np.ascontiguousarray(gh.reshape(EC, 128, VP))
    shared['genT_lo'] = np.ascontiguousarray((gpad - gh.astype(np.float32)).astype(np.float16).reshape(EC, 128, VP))

    in_maps = []
    for c in range(NCORES):
        m = dict(shared)
        sl = slice(c * BL, (c + 1) * BL)
        sq = seqs[sl].reshape(-1).astype(np.int16)
        m['seq_idx'] = np.tile(_wrap16(sq), (8, 1))
        cidx = (iso[sl] * 200 + dist[sl]).astype(np.int16)      # [BL, S, S] (q, k)
        bi = np.zeros((BL, 128, 512), np.int16)
        for b in range(BL):
            ct = np.ascontiguousarray(cidx[b].T).reshape(-1)     # k-major flat
            for g in range(8):
                bi[b, 16 * g:16 * g + 16] = _wrap16(ct[g * 8192:(g + 1) * 8192])
        m['bias_idx'] = bi
        m['memT'] = np.ascontiguousarray(
            memory[sl].transpose(2, 0, 1).reshape(E, BL * M)).astype(np.float16)
        in_maps.append(m)

    import os as _os
    res = run_bass_kernel_spmd(nc, in_maps, list(range(NCORES)),
                               trace=_os.environ.get("BASS_TRACE", "0") == "1")
    _last_res['res'] = res
    out = np.concatenate([res.results[c]['out'] for c in range(NCORES)], axis=0)
    return out.astype(np.float32)


if __name__ == "__main__":
    pass



# revision 1
# speedup vs baseline: 1.4429x; 1.4429x over previous
"""Trainium2 Bass kernel for nn_BaseDecoder (6-layer transformer decoder).

Sharding: data-parallel over batch, 8 NeuronCores x 4 batch elements.
Per-core layout: activations feature-major ("xT": [E partitions, tokens free]).
All matmuls fp16 w/ fp32 PSUM; layer-1 self-attn q/k/scores emulate fp32 via
hi/lo fp16 splits (scores there are ~N(0,590) and argmax-sensitive).
Attention scores are computed transposed ([k, q]) so the gathered relative
bias + causal mask (fp16, pre-scaled by 8, -480 masked fill) streams in
matching layout; softmax normalization: row-sum via ones-matmul -> reciprocal
on the [1, q] row -> PE broadcast -> multiplied into P before attn@V.
LayerNorm: partition sums via ones-matmuls, row math, PE broadcast, in-place.
"""
import sys
sys.path.insert(0, '/opt/trn_rl_repo')

import numpy as np
import concourse.bass as bass
import concourse.bacc as bacc
import concourse.mybir as mybir
import concourse.tile as tile
from concourse.bass_utils import run_bass_kernel_spmd
from contextlib import ExitStack

F32 = mybir.dt.float32
F16 = mybir.dt.float16
I16 = mybir.dt.int16
AF = mybir.ActivationFunctionType
ALU = mybir.AluOpType

B, S, M, E, H, F, L, V = 32, 256, 128, 1024, 16, 4096, 6, 200
DH = E // H
NCORES = 8
BL = B // NCORES
TOK = BL * S          # 1024
EC = E // 128         # 8
FC = F // 128         # 32
LN_EPS = 1e-5
MASK8 = -30000.0      # masked-entry fill (x8 units); kills exp even vs L1 max gap
VP = 256

_built = {}
_last_res = {}


def build_nc():
    nc = bacc.Bacc("TRN2", target_bir_lowering=False, debug=False)
    din = {}

    def inp(name, shape, dtype):
        din[name] = nc.dram_tensor(name, list(shape), dtype, kind="ExternalInput")

    inp("tokwT", (E, V), F32)
    inp("posencT", (E, S), F32)
    inp("seq_idx", (128, TOK // 16), I16)
    inp("bias_tab8", (128, 400), F32)
    inp("bias_idx", (BL, 128, 8192 // 16), I16)
    inp("bias_mask8", (128, 8192), F32)
    inp("mask_qk", (2, 128, S), F32)
    inp("identity", (128, 128), F32)
    inp("WqkvT", (L, 3 * EC, EC, 128, 128), F16)
    inp("Wqk_lo", (2 * EC, EC, 128, 128), F16)
    inp("WoT", (L, EC, EC, 128, 128), F16)
    inp("cWqkvT", (L, 3 * EC, EC, 128, 128), F16)
    inp("cWoT", (L, EC, EC, 128, 128), F16)
    inp("W1T", (L, FC, EC, 128, 128), F16)
    inp("W2T", (L, EC, FC, 128, 128), F16)
    inp("genT_hi", (EC, 128, VP), F16)
    inp("genT_lo", (EC, 128, VP), F16)
    inp("memT", (E, BL * M), F16)
    inp("WvT_mov", (L, 2, 128, EC * 512), F16)
    inp("cWvT_mov", (L, 2, 128, EC * 512), F16)
    out_t = nc.dram_tensor("out", [BL, S, V], F32, kind="ExternalOutput")
    bias_scr = nc.dram_tensor("bias_scr", [BL, 128, 8192], F16)

    with tile.TileContext(nc) as tc, ExitStack() as ctx:
        big = ctx.enter_context(tc.tile_pool(name="big", bufs=1))
        wpool = ctx.enter_context(tc.tile_pool(name="wp", bufs=2))
        sm = ctx.enter_context(tc.tile_pool(name="sm", bufs=1))
        ph = ctx.enter_context(tc.tile_pool(name="ph", bufs=2))   # per-head small tiles
        bias_p = ctx.enter_context(tc.tile_pool(name="biasp", bufs=2))
        wp2 = ctx.enter_context(tc.tile_pool(name="wp2", bufs=1))
        pgemm = ctx.enter_context(tc.tile_pool(name="pg", bufs=3, space="PSUM"))
        psT = ctx.enter_context(tc.tile_pool(name="psT", bufs=2, space="PSUM"))
        prow = ctx.enter_context(tc.tile_pool(name="prow", bufs=1, space="PSUM"))
        pbz = ctx.enter_context(tc.tile_pool(name="pbz", bufs=1, space="PSUM"))
        pout = ctx.enter_context(tc.tile_pool(name="pout", bufs=1, space="PSUM"))

        # ---------------- constants ----------------
        ident = big.tile([128, 128], F32, tag="ident")
        nc.sync.dma_start(ident[:], din["identity"][:])
        ones_col = big.tile([128, 1], F16, tag="ones_col")
        nc.vector.memset(ones_col[:], 1.0)
        ones_row = big.tile([1, 128], F16, tag="ones_row")
        nc.vector.memset(ones_row[:], 1.0)
        epsc = big.tile([128, 1], F32, tag="epsc")
        nc.vector.memset(epsc[:], LN_EPS)
        maskqk = big.tile([128, 2 * S], F32, tag="maskqk")
        nc.sync.dma_start(maskqk[:, 0:S], din["mask_qk"][0])
        nc.sync.dma_start(maskqk[:, S:2 * S], din["mask_qk"][1])
        memsb = big.tile([128, EC * 512], F16, tag="memsb")
        nc.sync.dma_start(memsb[:], din["memT"][:].rearrange("(ec p) t -> p ec t", p=128))

        # ---------------- embeddings ----------------
        A = big.tile([128, EC * TOK], F32, tag="A")
        tokw = big.tile([128, EC * V], F32, tag="qkA", name="tokw")
        nc.sync.dma_start(tokw[:], din["tokwT"][:].rearrange("(ec p) v -> p ec v", p=128))
        sidx = big.tile([128, TOK // 16], I16, tag="sidx")
        nc.sync.dma_start(sidx[:], din["seq_idx"][:])
        posenc = big.tile([128, EC * S], F32, tag="qkB", name="posenc")
        nc.sync.dma_start(posenc[:], din["posencT"][:].rearrange("(ec p) s -> p ec s", p=128))
        for ec in range(EC):
            nc.gpsimd.ap_gather(A[:, ec * TOK:(ec + 1) * TOK], tokw[:, ec * V:(ec + 1) * V],
                                sidx[:], channels=128, num_elems=V, d=1, num_idxs=TOK)
        for ec in range(EC):
            for b in range(BL):
                sl = A[:, ec * TOK + b * S: ec * TOK + (b + 1) * S]
                nc.vector.tensor_tensor(sl, sl, posenc[:, ec * S:(ec + 1) * S], op=ALU.add)

        # ---------------- bias build ----------------
        btab = big.tile([128, 400], F32, tag="btab")
        nc.sync.dma_start(btab[:], din["bias_tab8"][:])
        bmask = big.tile([128, 8192], F32, tag="qkB", name="bmask")
        nc.sync.dma_start(bmask[:], din["bias_mask8"][:])
        for b in range(BL):
            bidx = sm.tile([128, 512], I16, tag="bidx")
            nc.sync.dma_start(bidx[:], din["bias_idx"][b])
            graw = big.tile([128, 8192], F32, tag="qkA", name=f"graw{b}")
            nc.gpsimd.ap_gather(graw[:], btab[:], bidx[:], channels=128,
                                num_elems=400, d=1, num_idxs=8192)
            g16 = big.tile([128, 8192], F16, tag="vtok", name=f"g16_{b}")
            nc.vector.tensor_tensor(g16[:], graw[:], bmask[:], op=ALU.add)
            nc.sync.dma_start(bias_scr[b], g16[:])

        # -------------- persistent buffers --------------
        B16 = big.tile([128, EC * TOK], F16, tag="B16")

        _nn = [0]

        def _named(tag, shape, dtype):
            _nn[0] += 1
            return big.tile(shape, dtype, tag=tag, name=f"{tag}_{_nn[0]}")

        def new_qkA(dtype, n):
            return _named("qkA", [128, n], dtype)

        def new_qkB(dtype, n):
            return _named("qkB", [128, n], dtype)

        def new_alo():
            return _named("vtok", [128, EC * TOK], F16)

        def new_qcT():
            return _named("qkA", [128, EC * TOK], F16)

        def new_vtok():
            return _named("vtok", [128, EC * TOK], F16)

        # -------------- helpers --------------
        def hilo_row(dh_, dl_, src, n):
            nc.vector.tensor_copy(dh_[:, 0:n], src[:, 0:n])
            nc.vector.tensor_tensor(dl_[:, 0:n], src[:, 0:n], dh_[:, 0:n], op=ALU.subtract)

        def bcast_hilo(ps, rh, rl, n):
            nc.tensor.matmul(ps[:, 0:n], ones_row[:], rh[:, 0:n], start=True, stop=False)
            nc.tensor.matmul(ps[:, 0:n], ones_row[:], rl[:, 0:n], start=False, stop=True)

        def layernorm():
            """in-place LN of A; refresh B16."""
            a16 = _named("qkA", [128, EC * TOK], F16)
            sq = _named("vtok", [128, EC * TOK], F16)
            nc.vector.tensor_copy(a16[:], A[:])
            nc.scalar.activation(sq[:], A[:], AF.Square)
            negm = sm.tile([1, TOK], F32, tag="ln_negm")
            rr = sm.tile([1, TOK], F32, tag="ln_rr")
            for tkc in range(2):
                o = tkc * 512
                s1 = prow.tile([1, 512], F32, tag="row")
                for ec in range(EC):
                    nc.tensor.matmul(s1[:], ones_col[:], a16[:, ec * TOK + o: ec * TOK + o + 512],
                                     start=(ec == 0), stop=(ec == EC - 1))
                nc.scalar.activation(negm[:, o:o + 512], s1[:], AF.Copy, scale=-1.0 / E)
                s2 = prow.tile([1, 512], F32, tag="row")
                for ec in range(EC):
                    nc.tensor.matmul(s2[:], ones_col[:], sq[:, ec * TOK + o: ec * TOK + o + 512],
                                     start=(ec == 0), stop=(ec == EC - 1))
                v1 = sm.tile([1, 512], F32, tag="ln_v1")
                nc.scalar.activation(v1[:], s2[:], AF.Copy, scale=1.0 / E)
                m2 = sm.tile([1, 512], F32, tag="ln_m2")
                nc.vector.tensor_tensor(m2[:], negm[:, o:o + 512], negm[:, o:o + 512], op=ALU.mult)
                nc.vector.tensor_tensor(v1[:], v1[:], m2[:], op=ALU.subtract)
                sd = sm.tile([1, 512], F32, tag="ln_sd")
                nc.scalar.activation(sd[:], v1[:], AF.Sqrt, bias=epsc[0:1, :])
                nc.vector.reciprocal(rr[:, o:o + 512], sd[:])
            nmh = sm.tile([1, TOK], F16, tag="ln_nmh")
            rrh = sm.tile([1, TOK], F16, tag="ln_rrh")
            nc.vector.tensor_copy(nmh[:], negm[:])
            nc.vector.tensor_copy(rrh[:], rr[:])
            for tkc in range(2):
                o = tkc * 512
                mb = pgemm.tile([128, 512], F32, tag="g")
                rb = pgemm.tile([128, 512], F32, tag="g")
                nc.tensor.matmul(mb[:], ones_row[:], nmh[:, o:o + 512])
                nc.tensor.matmul(rb[:], ones_row[:], rrh[:, o:o + 512])
                for ec in range(EC):
                    sl = A[:, ec * TOK + o: ec * TOK + o + 512]
                    nc.vector.tensor_tensor(sl, sl, mb[:], op=ALU.add)
                    nc.vector.tensor_tensor(sl, sl, rb[:], op=ALU.mult)
                    nc.vector.tensor_copy(B16[:, ec * TOK + o: ec * TOK + o + 512], sl)

        def gemm_oc_tok(dst, wdram, l_idx, octile0, n_octiles, mov, mov_lo=None,
                        w_lo=None, wlo_octile0=0, dst_hilo=False, dst_off=0):
            """dst[oc_tile*TOK + tok] = W.x ; stat = weight tiles, mov feature-major."""
            for mt in range(n_octiles):
                wt = wpool.tile([128, EC * 128], F16, tag="wload")
                src = wdram[l_idx, octile0 + mt] if l_idx is not None else wdram[octile0 + mt]
                nc.sync.dma_start(wt[:], src.rearrange("kc a b -> a kc b"))
                wlt = None
                if w_lo is not None:
                    wlt = wp2.tile([128, EC * 128], F16, tag="w2load")
                    nc.sync.dma_start(wlt[:], w_lo[wlo_octile0 + mt].rearrange("kc a b -> a kc b"))
                for tkc in range(2):
                    o = tkc * 512
                    ps = pgemm.tile([128, 512], F32, tag="g")
                    nmm = EC * (3 if w_lo is not None else 1)
                    i = 0
                    for kc in range(EC):
                        mv = mov[:, kc * TOK + o: kc * TOK + o + 512]
                        nc.tensor.matmul(ps[:], wt[:, kc * 128:(kc + 1) * 128], mv,
                                         start=(i == 0), stop=(i == nmm - 1)); i += 1
                        if w_lo is not None:
                            mvl = mov_lo[:, kc * TOK + o: kc * TOK + o + 512]
                            nc.tensor.matmul(ps[:], wt[:, kc * 128:(kc + 1) * 128], mvl,
                                             start=False, stop=(i == nmm - 1)); i += 1
                            nc.tensor.matmul(ps[:], wlt[:, kc * 128:(kc + 1) * 128], mv,
                                             start=False, stop=(i == nmm - 1)); i += 1
                    if dst_hilo:
                        hi_sl = dst[:, mt * TOK + o: mt * TOK + o + 512]
                        lo_sl = dst[:, 8192 + mt * TOK + o: 8192 + mt * TOK + o + 512]
                        nc.vector.tensor_copy(hi_sl, ps[:])
                        nc.vector.tensor_tensor(lo_sl, ps[:], hi_sl, op=ALU.subtract)
                    else:
                        nc.vector.tensor_copy(dst[:, dst_off + mt * TOK + o: dst_off + mt * TOK + o + 512], ps[:])

        def residual_gemm(wdram, l_idx, mov):
            """A += W.mov  (Wo / cWo / ffn2-style: E out-tiles)"""
            for mt in range(EC):
                wt = wpool.tile([128, EC * 128], F16, tag="wload")
                nc.sync.dma_start(wt[:], wdram[l_idx, mt].rearrange("kc a b -> a kc b"))
                for tkc in range(2):
                    o = tkc * 512
                    ps = pgemm.tile([128, 512], F32, tag="g")
                    for kc in range(EC):
                        nc.tensor.matmul(ps[:], wt[:, kc * 128:(kc + 1) * 128],
                                         mov[:, kc * TOK + o: kc * TOK + o + 512],
                                         start=(kc == 0), stop=(kc == EC - 1))
                    sl = A[:, mt * TOK + o: mt * TOK + o + 512]
                    nc.vector.tensor_tensor(sl, sl, ps[:], op=ALU.add)

        # ================== layers ==================
        for l in range(L):
            first = (l == 0)
            # ---------- self-attention: q/k/v projections ----------
            if first:
                XHI = B16
                XLO = new_alo()
                nc.vector.tensor_copy(XHI[:], A[:])
                nc.vector.tensor_tensor(XLO[:], A[:], XHI[:], op=ALU.subtract)
                qT = new_qkA(F16, 2 * EC * TOK)
                kT = new_qkB(F16, 2 * EC * TOK)
                gemm_oc_tok(qT, din["WqkvT"], 0, 0, EC, XHI, mov_lo=XLO,
                            w_lo=din["Wqk_lo"], wlo_octile0=0, dst_hilo=True)
                gemm_oc_tok(kT, din["WqkvT"], 0, EC, EC, XHI, mov_lo=XLO,
                            w_lo=din["Wqk_lo"], wlo_octile0=EC, dst_hilo=True)
            else:
                qT = new_qkA(F16, EC * TOK)
                kT = new_qkB(F16, EC * TOK)
                gemm_oc_tok(qT, din["WqkvT"], l, 0, EC, B16)
                gemm_oc_tok(kT, din["WqkvT"], l, EC, EC, B16)
            # v gemm: out [tok, oc]; stat = B16 token tiles, mov = WvT columns
            VT = new_vtok()
            for occ in range(2):
                wv = wpool.tile([128, EC * 512], F16, tag="wvload")
                nc.sync.dma_start(wv[:], din["WvT_mov"][l, occ])
                for tt in range(EC):
                    ps = pgemm.tile([128, 512], F32, tag="g")
                    for kc in range(EC):
                        nc.tensor.matmul(ps[:], B16[:, kc * TOK + tt * 128: kc * TOK + tt * 128 + 128],
                                         wv[:, kc * 512:(kc + 1) * 512],
                                         start=(kc == 0), stop=(kc == EC - 1))
                    nc.vector.tensor_copy(VT[:, tt * E + occ * 512: tt * E + occ * 512 + 512], ps[:])

            # ---------- L1: per-(bh,qc) masked max ----------
            if first:
                negMb0 = sm.tile([128, 64], F32, tag="negMb0")
                negMb1 = sm.tile([128, 64], F32, tag="negMb1")
                negMb = [negMb0, negMb1]
                for b in range(BL):
                    for h in range(H):
                        bh = b * H + h
                        e2, off = h // 2, (h % 2) * 64
                        qh = qT[off:off + 64, e2 * TOK + b * S: e2 * TOK + (b + 1) * S]
                        ql = qT[off:off + 64, 8192 + e2 * TOK + b * S: 8192 + e2 * TOK + (b + 1) * S]
                        kh = kT[off:off + 64, e2 * TOK + b * S: e2 * TOK + (b + 1) * S]
                        kl = kT[off:off + 64, 8192 + e2 * TOK + b * S: 8192 + e2 * TOK + (b + 1) * S]
                        for qc in range(2):
                            ps = psT.tile([128, S], F32, tag="sT")
                            nc.tensor.matmul(ps[:], qh[:, qc * 128:(qc + 1) * 128], kh[:],
                                             start=True, stop=False)
                            nc.tensor.matmul(ps[:], qh[:, qc * 128:(qc + 1) * 128], kl[:],
                                             start=False, stop=False)
                            nc.tensor.matmul(ps[:], ql[:, qc * 128:(qc + 1) * 128], kh[:],
                                             start=False, stop=True)
                            scr = ph.tile([128, S], F32, tag="ttr_scr")
                            nc.vector.tensor_tensor(scr[:], ps[:],
                                                    maskqk[:, qc * S:(qc + 1) * S],
                                                    op=ALU.add)
                            nc.vector.tensor_reduce(negMb[qc][:, bh:bh + 1], scr[:],
                                                    axis=mybir.AxisListType.X,
                                                    op=ALU.max)
                negMT = sm.tile([64, S], F32, tag="negMT")
                for qc in range(2):
                    pt = pout.tile([64, 256], F32, tag="aout")
                    nc.tensor.transpose(pt[0:64, 0:128], negMb[qc][:], ident[:])
                    nc.vector.tensor_copy(negMT[:, qc * 128:(qc + 1) * 128], pt[0:64, 0:128])
                negMTh2 = sm.tile([64, 256], F16, tag="negMTh2")
                negMTl2 = sm.tile([64, 256], F16, tag="negMTl2")
                hilo_row(negMTh2, negMTl2, negMT, 256)

            # ---------- self-attention core ----------
            AO = B16   # attn output overwrites B16 (last gemm consumer done)
            for b in range(BL):
                for h in range(H):
                    bh = b * H + h
                    e2, off = h // 2, (h % 2) * 64
                    qsl = qT[off:off + 64, e2 * TOK + b * S: e2 * TOK + (b + 1) * S]
                    ksl = kT[off:off + 64, e2 * TOK + b * S: e2 * TOK + (b + 1) * S]
                    btile = bias_p.tile([128, 512], F16, tag="bias")
                    for kc in range(2):
                        src = bias_scr[b, 64 * kc + h: 64 * kc + h + 49: 16, :]
                        nc.sync.dma_start(
                            btile[:, kc * S:(kc + 1) * S],
                            src.rearrange("g (k q) -> g k q", q=S))
                    if first:
                        nmrh = ph.tile([1, S], F16, tag="nmrh")
                        nmrl = ph.tile([1, S], F16, tag="nmrl")
                        nc.sync.dma_start(nmrh[:], negMTh2[bh:bh + 1, :])
                        nc.sync.dma_start(nmrl[:], negMTl2[bh:bh + 1, :])
                        qh = qT[off:off + 64, e2 * TOK + b * S: e2 * TOK + (b + 1) * S]
                        ql = qT[off:off + 64, 8192 + e2 * TOK + b * S: 8192 + e2 * TOK + (b + 1) * S]
                        kh = kT[off:off + 64, e2 * TOK + b * S: e2 * TOK + (b + 1) * S]
                        kl = kT[off:off + 64, 8192 + e2 * TOK + b * S: 8192 + e2 * TOK + (b + 1) * S]
                        bz = pbz.tile([128, S], F32, tag="bz")
                        bcast_hilo(bz, nmrh[:], nmrl[:], S)
                    PT = ph.tile([128, 2 * S], F16, tag="PT")
                    for kc in range(2):
                        ps = psT.tile([128, S], F32, tag="sT")
                        if first:
                            nc.tensor.matmul(ps[:], kh[:, kc * 128:(kc + 1) * 128], qh[:],
                                             start=True, stop=False)
                            nc.tensor.matmul(ps[:], kh[:, kc * 128:(kc + 1) * 128], ql[:],
                                             start=False, stop=False)
                            nc.tensor.matmul(ps[:], kl[:, kc * 128:(kc + 1) * 128], qh[:],
                                             start=False, stop=True)
                        else:
                            nc.tensor.matmul(ps[:], ksl[:, kc * 128:(kc + 1) * 128], qsl)
                        t1 = ph.tile([128, S], F32 if first else F16, tag="t1")
                        nc.vector.tensor_tensor(t1[:], ps[:], btile[:, kc * S:(kc + 1) * S],
                                                op=ALU.add)
                        if first:
                            nc.vector.tensor_tensor(t1[:], t1[:], bz[:], op=ALU.subtract)
                        nc.scalar.activation(PT[:, kc * S:(kc + 1) * S], t1[:], AF.Exp,
                                             scale=0.125)
                    zr = prow.tile([1, S], F32, tag="row")
                    for kc in range(2):
                        nc.tensor.matmul(zr[:], ones_col[:], PT[:, kc * S:(kc + 1) * S],
                                         start=(kc == 0), stop=(kc == 1))
                    rz = ph.tile([1, S], F32, tag="rz")
                    nc.vector.reciprocal(rz[:], zr[:])
                    rzh = ph.tile([1, S], F16, tag="rzh")
                    rzl = ph.tile([1, S], F16, tag="rzl")
                    hilo_row(rzh, rzl, rz, S)
                    zb = pbz.tile([128, S], F32, tag="bz")
                    bcast_hilo(zb, rzh, rzl, S)
                    po = pout.tile([64, S], F32, tag="aout")
                    for kc in range(2):
                        pn = ph.tile([128, S], F16, tag="pn")
                        nc.vector.tensor_tensor(pn[:], PT[:, kc * S:(kc + 1) * S], zb[:],
                                                op=ALU.mult)
                        nc.tensor.matmul(po[:], VT[:, (2 * b + kc) * E + h * 64: (2 * b + kc) * E + h * 64 + 64],
                                         pn[:], start=(kc == 0), stop=(kc == 1))
                    nc.vector.tensor_copy(
                        AO[(h % 2) * 64:(h % 2) * 64 + 64, (h // 2) * TOK + b * S:(h // 2) * TOK + (b + 1) * S],
                        po[:])
            residual_gemm(din["WoT"], l, AO)
            layernorm()

            # ---------- cross-attention ----------
            qcT = new_qcT()
            gemm_oc_tok(qcT, din["cWqkvT"], l, 0, EC, B16)
            KV = new_vtok()     # [:, :4096] = kcT (oc x bm), [:, 4096:] = vc (bm x oc)
            for mt in range(EC):
                wt = wpool.tile([128, EC * 128], F16, tag="wload")
                nc.sync.dma_start(wt[:], din["cWqkvT"][l, EC + mt].rearrange("kc a b -> a kc b"))
                ps = pgemm.tile([128, 512], F32, tag="g")
                for kc in range(EC):
                    nc.tensor.matmul(ps[:], wt[:, kc * 128:(kc + 1) * 128],
                                     memsb[:, kc * 512:(kc + 1) * 512],
                                     start=(kc == 0), stop=(kc == EC - 1))
                nc.vector.tensor_copy(KV[:, mt * 512:(mt + 1) * 512], ps[:])
            for occ in range(2):
                wv = wpool.tile([128, EC * 512], F16, tag="wvload", name=f"cwv_{l}_{occ}")
                nc.sync.dma_start(wv[:], din["cWvT_mov"][l, occ])
                for bt in range(BL):
                    ps = pgemm.tile([128, 512], F32, tag="g")
                    for kc in range(EC):
                        nc.tensor.matmul(ps[:], memsb[:, kc * 512 + bt * 128: kc * 512 + bt * 128 + 128],
                                         wv[:, kc * 512:(kc + 1) * 512],
                                         start=(kc == 0), stop=(kc == EC - 1))
                    nc.vector.tensor_copy(KV[:, 4096 + bt * 1024 + occ * 512: 4096 + bt * 1024 + occ * 512 + 512],
                                          ps[:])
            AO = B16
            for b in range(BL):
                for h in range(H):
                    e2, off = h // 2, (h % 2) * 64
                    ps = psT.tile([128, S], F32, tag="sT")
                    nc.tensor.matmul(ps[:], KV[off:off + 64, e2 * 512 + b * 128: e2 * 512 + (b + 1) * 128],
                                     qcT[off:off + 64, e2 * TOK + b * S: e2 * TOK + (b + 1) * S])
                    Ec = ph.tile([128, S], F16, tag="Ec")
                    nc.scalar.activation(Ec[:], ps[:], AF.Exp, scale=0.125)
                    zr = prow.tile([1, S], F32, tag="row")
                    nc.tensor.matmul(zr[:], ones_col[:], Ec[:])
                    rz = ph.tile([1, S], F32, tag="rz")
                    nc.vector.reciprocal(rz[:], zr[:])
                    rzh = ph.tile([1, S], F16, tag="rzh")
                    rzl = ph.tile([1, S], F16, tag="rzl")
                    hilo_row(rzh, rzl, rz, S)
                    zb = pbz.tile([128, S], F32, tag="bz")
                    bcast_hilo(zb, rzh, rzl, S)
                    pn = ph.tile([128, S], F16, tag="pn")
                    nc.vector.tensor_tensor(pn[:], Ec[:], zb[:], op=ALU.mult)
                    po = pout.tile([64, S], F32, tag="aout")
                    nc.tensor.matmul(po[:], KV[:, 4096 + b * 1024 + h * 64: 4096 + b * 1024 + h * 64 + 64],
                                     pn[:])
                    nc.vector.tensor_copy(
                        AO[off:off + 64, e2 * TOK + b * S: e2 * TOK + (b + 1) * S], po[:])
            residual_gemm(din["cWoT"], l, AO)
            layernorm()

            # ---------- FFN ----------
            h1a = new_qkA(F16, 16 * TOK)
            h1b = new_qkB(F16, 16 * TOK)

            def h1sl(fc, o):
                t = h1a if fc < 16 else h1b
                return t[:, (fc % 16) * TOK + o: (fc % 16) * TOK + o + 512]

            for fc in range(FC):
                wt = wpool.tile([128, EC * 128], F16, tag="wload")
                nc.sync.dma_start(wt[:], din["W1T"][l, fc].rearrange("kc a b -> a kc b"))
                for tkc in range(2):
                    o = tkc * 512
                    ps = pgemm.tile([128, 512], F32, tag="g")
                    for kc in range(EC):
                        nc.tensor.matmul(ps[:], wt[:, kc * 128:(kc + 1) * 128],
                                         B16[:, kc * TOK + o: kc * TOK + o + 512],
                                         start=(kc == 0), stop=(kc == EC - 1))
                    nc.scalar.activation(h1sl(fc, o), ps[:], AF.Gelu)
            for mt in range(EC):
                w2a = wp2.tile([128, 16 * 128], F16, tag="w2load", name=f"w2a_{l}_{mt}")
                nc.sync.dma_start(w2a[:], din["W2T"][l, mt, 0:16].rearrange("kc a b -> a kc b"))
                w2b = wp2.tile([128, 16 * 128], F16, tag="w2loadb", name=f"w2b_{l}_{mt}")
                nc.sync.dma_start(w2b[:], din["W2T"][l, mt, 16:32].rearrange("kc a b -> a kc b"))
                for tkc in range(2):
                    o = tkc * 512
                    ps = pgemm.tile([128, 512], F32, tag="g")
                    for fc in range(FC):
                        w2t = w2a if fc < 16 else w2b
                        nc.tensor.matmul(ps[:], w2t[:, (fc % 16) * 128:((fc % 16) + 1) * 128],
                                         h1sl(fc, o),
                                         start=(fc == 0), stop=(fc == FC - 1))
                    sl = A[:, mt * TOK + o: mt * TOK + o + 512]
                    nc.vector.tensor_tensor(sl, sl, ps[:], op=ALU.add)
            layernorm()

        # ---------------- final LN + generator ----------------
        layernorm()
        XLO = new_alo()
        nc.vector.tensor_tensor(XLO[:], A[:], B16[:], op=ALU.subtract)
        genh = _named("qkA", [128, EC * VP], F16)
        genl = _named("qkB", [128, EC * VP], F16)
        nc.sync.dma_start(genh[:], din["genT_hi"][:].rearrange("ec a b -> a ec b"))
        nc.sync.dma_start(genl[:], din["genT_lo"][:].rearrange("ec a b -> a ec b"))
        for tt in range(EC):
            ps = pgemm.tile([128, 512], F32, tag="g")
            n3 = 3 * EC
            i = 0
            for kc in range(EC):
                sth = B16[:, kc * TOK + tt * 128: kc * TOK + tt * 128 + 128]
                stl = XLO[:, kc * TOK + tt * 128: kc * TOK + tt * 128 + 128]
                mvh = genh[:, kc * VP:(kc + 1) * VP]
                mvl = genl[:, kc * VP:(kc + 1) * VP]
                nc.tensor.matmul(ps[:, 0:VP], sth, mvh, start=(i == 0), stop=(i == n3 - 1)); i += 1
                nc.tensor.matmul(ps[:, 0:VP], sth, mvl, start=False, stop=(i == n3 - 1)); i += 1
                nc.tensor.matmul(ps[:, 0:VP], stl, mvh, start=False, stop=(i == n3 - 1)); i += 1
            osb = bias_p.tile([128, VP], F32, tag="bias")
            nc.vector.tensor_copy(osb[:], ps[:, 0:VP])
            b0, s0 = (tt * 128) // S, (tt * 128) % S
            nc.sync.dma_start(out_t[b0, s0:s0 + 128, 0:V], osb[:, 0:V])

    nc.compile()
    return nc


# ================= host side =================

def _posenc_np():
    den = np.exp(-np.arange(0, E, 2, dtype=np.float32) *
                 np.float32(np.log(10000.0)) / np.float32(E)).astype(np.float32)
    pos = np.arange(S, dtype=np.float32)[:, None]
    pe = np.zeros((S, E), np.float32)
    pe[:, 0::2] = np.sin(pos * den)
    pe[:, 1::2] = np.cos(pos * den)
    return pe


def _tile_w(wT, dtype=np.float16):
    """[K, Mo] -> [Mo/128, K/128, 128, 128]"""
    K, Mo = wT.shape
    return np.ascontiguousarray(
        wT.reshape(K // 128, 128, Mo // 128, 128).transpose(2, 0, 1, 3)).astype(dtype)


def _wrap16(flat):
    return np.ascontiguousarray(flat.reshape(-1, 16).T)


def kernel(**inputs):
    inputs = {k: np.asarray(v) for k, v in inputs.items()}
    seqs = inputs['sequences'].astype(np.int64)
    dist = inputs['distance_squares'].astype(np.int64)
    iso = inputs['isopen_squares'].astype(np.int64)
    memory = inputs['memory'].astype(np.float32)
    tok_w = inputs['tok_emb_w'].astype(np.float32)
    dist_w = inputs['dist_emb_w'].astype(np.float32)
    iso_w = inputs['iso_emb_w'].astype(np.float32)

    if 'nc' not in _built:
        _built['nc'] = build_nc()
    nc = _built['nc']

    # ---- shared (replicated) host tensors ----
    shared = {}
    shared['tokwT'] = np.ascontiguousarray((tok_w * np.float32(np.sqrt(E))).T)
    shared['posencT'] = np.ascontiguousarray(_posenc_np().T)
    tab = np.concatenate([dist_w + iso_w[0], dist_w + iso_w[1]], axis=0)  # [400, 16]
    shared['bias_tab8'] = np.tile(np.ascontiguousarray((8.0 * tab).T), (8, 1)).astype(np.float32)
    # bias mask in gather layout: row 16g+h covers j = g*8192 + i, j = k*256+q
    jj = (np.arange(8)[:, None] * 8192 + np.arange(8192)[None, :])  # [8, 8192]
    kk, qq = jj // S, jj % S
    mrow = np.where(kk > qq, np.float32(MASK8), np.float32(0.0))    # [8, 8192]
    shared['bias_mask8'] = np.repeat(mrow, 16, axis=0).astype(np.float32)
    mq = np.zeros((2, 128, S), np.float32)
    for qc in range(2):
        qv = qc * 128 + np.arange(128)[:, None]
        mq[qc] = np.where(np.arange(S)[None, :] > qv, np.float32(-1e30), np.float32(0.0))
    shared['mask_qk'] = mq
    shared['identity'] = np.eye(128, dtype=np.float32)

    Wqkv_s = inputs['Wqkv_s'].astype(np.float32)
    shared['WqkvT'] = np.stack([_tile_w(Wqkv_s[l].T) for l in range(L)])
    qkT0 = Wqkv_s[0, :2 * E].T  # [E, 2E] f32
    hi = qkT0.astype(np.float16)
    shared['Wqk_lo'] = _tile_w((qkT0 - hi.astype(np.float32)))
    shared['WoT'] = np.stack([_tile_w(inputs['Wo_s'][l].T) for l in range(L)])
    Wqkv_c = inputs['Wqkv_c'].astype(np.float32)
    shared['cWqkvT'] = np.stack([_tile_w(Wqkv_c[l].T) for l in range(L)])
    shared['cWoT'] = np.stack([_tile_w(inputs['Wo_c'][l].T) for l in range(L)])
    def _vmov(Wqkv_f32):
        out = np.zeros((L, 2, 128, EC * 512), np.float16)
        for l in range(L):
            WvT = Wqkv_f32[l, 2 * E:3 * E].T.astype(np.float16)
            for occ in range(2):
                out[l, occ] = WvT.reshape(EC, 128, E)[:, :, occ * 512:(occ + 1) * 512]\
                    .transpose(1, 0, 2).reshape(128, EC * 512)
        return out
    shared['WvT_mov'] = _vmov(Wqkv_s)
    shared['cWvT_mov'] = _vmov(Wqkv_c)
    shared['W1T'] = np.stack([_tile_w(inputs['W1'][l].T) for l in range(L)])
    shared['W2T'] = np.stack([_tile_w(inputs['W2'][l].T) for l in range(L)])
    gpad = np.zeros((E, VP), np.float32)
    gpad[:, :V] = inputs['gen_w'].astype(np.float32).T
    gh = gpad.astype(np.float16)
    shared['genT_hi'] = np.ascontiguousarray(gh.reshape(EC, 128, VP))
    shared['genT_lo'] = np.ascontiguousarray((gpad - gh.astype(np.float32)).astype(np.float16).reshape(EC, 128, VP))

    in_maps = []
    for c in range(NCORES):
        m = dict(shared)
        sl = slice(c * BL, (c + 1) * BL)
        sq = seqs[sl].reshape(-1).astype(np.int16)
        m['seq_idx'] = np.tile(_wrap16(sq), (8, 1))
        cidx = (iso[sl] * 200 + dist[sl]).astype(np.int16)      # [BL, S, S] (q, k)
        bi = np.zeros((BL, 128, 512), np.int16)
        for b in range(BL):
            ct = np.ascontiguousarray(cidx[b].T).reshape(-1)     # k-major flat
            for g in range(8):
                bi[b, 16 * g:16 * g + 16] = _wrap16(ct[g * 8192:(g + 1) * 8192])
        m['bias_idx'] = bi
        m['memT'] = np.ascontiguousarray(
            memory[sl].transpose(2, 0, 1).reshape(E, BL * M)).astype(np.float16)
        in_maps.append(m)

    import os as _os
    res = run_bass_kernel_spmd(nc, in_maps, list(range(NCORES)),
                               trace=_os.environ.get("BASS_TRACE", "0") == "1")
    _last_res['res'] = res
    out = np.concatenate([res.results[c]['out'] for c in range(NCORES)], axis=0)
    return out.astype(np.float32)


if __name__ == "__main__":
    pass

